# revision 5
# baseline (speedup 1.0000x reference)
# Causal self-attention kernel for 8 Trainium2 NeuronCores — bf16 edition.
#
# Problem (hardcoded): B=2, S=2048, D=1024, H=16 heads of dk=64.
#   q,k,v = x @ W.T + b (torch Linear), per-head causal softmax attention,
#   out[b,s,:] = concat_h(attn_h @ v_h). No output projection.
#
# Sharding: 8 cores = 2 batches x 4 head-groups. Core c handles batch c//4
# and heads [4*(c%4), 4*(c%4)+4) => output channels [256*(c%4), +256).
# No cross-device communication.
#
# Design notes (engine-roofline driven; rel err ~4e-3 vs fp32 reference):
#   - Everything on the PE streams bf16 (x, W, xT, qT/kT, v, attention
#     weights). bf16 matmuls run 1 cycle/row at ANY moving width, so the
#     diagonal score/PV pieces run at their exact widths (f32r needed
#     >=256-wide padding + memset), bf16 weights enable fast-weight-load,
#     and PE transposes drop 1.5 -> 1.0 cycles/row. All PSUM accumulation
#     stays fp32 (matmul outputs; softmax denominators via a ones column
#     in the augmented v).
#   - x is shipped bf16 from the host (halves the dominant DMA load);
#     out is returned bf16 and upcast on the host.
#   - The causal mask and the transpose identity are generated on-chip
#     (gpsimd memset + affine_select): nothing but x blocks the first
#     PE transpose.
#   - Score matmuls of a head pair pack onto disjoint PE row-quadrants
#     (partitions 0:64 / 64:128) and run concurrently.
#   - The PV accumulator is evacuated once per unit AFTER its last
#     accumulation: early per-block drains serialized against every
#     following PV matmul into the same PSUM banks (~0.5us PE stall each).
#     The per-block transpose/normalize/DMA finish-work is deferred into
#     the NEXT unit's filler list so its 8 pot transposes don't
#     monopolize the 2-slot pp PSUM pool at the unit boundary (and the
#     final famine-bound unit inherits real work).
#   - Projection/transpose filler work is drip-fed into the attention
#     phase as late as each piece's consumer allows, so the big late
#     attention units (which carry most of the exp load) keep the PE fed.
#   - reps: emits the whole kernel body `reps` times in one NEFF for the
#     slope-based timing harness. kernel() itself uses reps=1.

import numpy as np
import ml_dtypes

BF = ml_dtypes.bfloat16

B, S, D, H = 2, 2048, 1024, 16
DK = D // H            # 64
NCORES = 8
HPC = 4                # heads per core
E = HPC * DK           # 256 output channels per core
EA = HPC * (DK + 1)    # 260 augmented v width (ones col per head)
P = 128
NSB = S // P           # 16 s-blocks
NDC = D // P           # 8 d-chunks
CW = 512               # attention sq-chunk width
NCH = S // CW          # 4 chunks

_cache = {}


def _build_module(reps=1):
    from contextlib import ExitStack

    import concourse.bacc as bacc
    import concourse.mybir as mybir
    import concourse.tile as tile

    f32 = mybir.dt.float32
    bf16 = mybir.dt.bfloat16

    nc = bacc.Bacc("TRN2", target_bir_lowering=False, debug=False)

    x_d = nc.dram_tensor("x", [S, D], bf16, kind="ExternalInput")
    wq_d = nc.dram_tensor("wq_t", [D, E], bf16, kind="ExternalInput")
    wk_d = nc.dram_tensor("wk_t", [D, E], bf16, kind="ExternalInput")
    wv_d = nc.dram_tensor("wv_t", [D, EA], bf16, kind="ExternalInput")
    bq_d = nc.dram_tensor("bq", [1, E], f32, kind="ExternalInput")
    bk_d = nc.dram_tensor("bk", [1, E], f32, kind="ExternalInput")
    bv_d = nc.dram_tensor("bv", [P, EA], f32, kind="ExternalInput")
    out_d = nc.dram_tensor("out", [S, E], bf16, kind="ExternalOutput")

    with tile.TileContext(nc) as tc:
        for rep in range(reps):
            with ExitStack() as ctx:
                _build_rep(nc, tc, ctx, tile, mybir, rep,
                           x_d, wq_d, wk_d, wv_d, bq_d, bk_d, bv_d,
                           out_d)

    nc.compile()
    return nc


def _build_rep(nc, tc, ctx, tile, mybir, rep,
               x_d, wq_d, wk_d, wv_d, bq_d, bk_d, bv_d,
               out_d):
    f32 = mybir.dt.float32
    bf16 = mybir.dt.bfloat16
    Exp = mybir.ActivationFunctionType.Exp

    consts = ctx.enter_context(tc.tile_pool(name=f"consts{rep}", bufs=1))
    qkv = ctx.enter_context(tc.tile_pool(name=f"qkv{rep}", bufs=1))
    outst = ctx.enter_context(tc.tile_pool(name=f"outst{rep}", bufs=1))
    xin = ctx.enter_context(tc.tile_pool(name=f"xin{rep}", bufs=4))
    xtp = ctx.enter_context(tc.tile_pool(name=f"xt{rep}", bufs=1))
    pp = ctx.enter_context(tc.tile_pool(name=f"pp{rep}", bufs=2, space="PSUM"))
    pscp = ctx.enter_context(
        tc.tile_pool(name=f"psc{rep}", bufs=2, space="PSUM"))
    paccp = ctx.enter_context(
        tc.tile_pool(name=f"pacc{rep}", bufs=1, space="PSUM"))
    attnp = ctx.enter_context(tc.tile_pool(name=f"attn{rep}", bufs=3))
    otnp = ctx.enter_context(tc.tile_pool(name=f"otn{rep}", bufs=2))

    # ---- constants ----
    wq_sb = consts.tile([P, NDC, E], bf16, tag="wq")
    wk_sb = consts.tile([P, NDC, E], bf16, tag="wk")
    wv_sb = consts.tile([P, NDC, EA], bf16, tag="wv")
    bqc_sb = consts.tile([P, 2], f32, tag="bqc")
    bkc_sb = consts.tile([P, 2], f32, tag="bkc")
    bv_sb = consts.tile([P, EA], f32, tag="bv")
    mask_sb = consts.tile([P, P], bf16, tag="mask")
    ident_sb = consts.tile([P, P], bf16, tag="ident")
    ones_sb = consts.tile([P, P], bf16, tag="ones")

    x_tiles = {}

    def emit_x_dma(sb):
        # two half-width DMAs: the dcg-0 transposes only need cols 0:512,
        # so they start as soon as the first half lands
        x_tile = xin.tile([P, D], bf16, tag="x")
        nc.sync.dma_start(
            out=x_tile[:, 0:D // 2], in_=x_d[sb * P:(sb + 1) * P, 0:D // 2]
        )
        nc.sync.dma_start(
            out=x_tile[:, D // 2:D], in_=x_d[sb * P:(sb + 1) * P, D // 2:D]
        )
        x_tiles[sb] = x_tile

    # identity (for PE transposes) and the causal 0/1 mask are generated
    # on-chip on the idle Pool engine: no DMA on the critical prologue
    # path, and the first transpose no longer waits on a DGE round-trip.
    nc.gpsimd.memset(ones_sb, 1.0)
    nc.gpsimd.affine_select(
        ident_sb, ones_sb, pattern=[[-1, P]], base=0,
        channel_multiplier=1, compare_op=mybir.AluOpType.is_equal, fill=0.0,
    )
    nc.gpsimd.affine_select(
        mask_sb, ones_sb, pattern=[[1, P]], base=0,
        channel_multiplier=-1, compare_op=mybir.AluOpType.is_ge, fill=0.0,
    )
    # weights/biases stream on the Activation HWDGE queue (idle until the
    # first exp anyway) while x tiles stream on the SP queue.
    nc.scalar.dma_start(out=wq_sb, in_=wq_d[:].rearrange("(c p) e -> p c e", p=P))
    nc.scalar.dma_start(out=wk_sb, in_=wk_d[:].rearrange("(c p) e -> p c e", p=P))
    nc.scalar.dma_start(out=bqc_sb, in_=bq_d[:].rearrange("o (c p) -> p (o c)", p=P))
    nc.scalar.dma_start(out=bkc_sb, in_=bk_d[:].rearrange("o (c p) -> p (o c)", p=P))
    for _sb in range(4):
        emit_x_dma(_sb)
    nc.sync.dma_start(out=wv_sb, in_=wv_d[:].rearrange("(c p) e -> p c e", p=P))
    nc.sync.dma_start(out=bv_sb, in_=bv_d[:])

    qT = qkv.tile([P, 2, S], bf16, tag="qT")
    kT = qkv.tile([P, 2, S], bf16, tag="kT")
    v_sb = qkv.tile([P, NSB, EA], bf16, tag="v")
    out_sb = outst.tile([P, NSB, E], bf16, tag="out")
    # xT: [d%128, s-block, d-chunk, 128] so transpose copies are
    # contiguous 512-wide (d-chunk groups of 4)
    xT = xtp.tile([P, NSB, NDC, P], bf16, tag="xT")

    def emit_xt(sb, dcg):
        # transpose 4 d-chunks of x block sb into one psum tile
        if dcg == 0 and sb not in x_tiles:
            emit_x_dma(sb)
        ptile = pp.tile([P, 512], bf16, tag="pp")
        for k in range(4):
            dc = dcg * 4 + k
            nc.tensor.transpose(
                ptile[:, k * P:(k + 1) * P],
                x_tiles[sb][:, dc * P:(dc + 1) * P],
                ident_sb,
            )
        dst = xT[:, sb, dcg * 4:(dcg + 1) * 4, :]
        nc.vector.tensor_copy(dst, ptile)

    def emit_qk_proj(which, eb, sc):
        w_sb = wq_sb if which == 0 else wk_sb
        bc = bqc_sb if which == 0 else bkc_sb
        dst = qT if which == 0 else kT
        ps = pp.tile([P, 512], f32, tag="pp")
        sb0 = sc * 512 // P
        for dc in range(NDC):
            nc.tensor.matmul(
                ps,
                lhsT=w_sb[:, dc, eb * P:(eb + 1) * P],
                rhs=xT[:, sb0:sb0 + 4, dc, :],
                start=(dc == 0),
                stop=(dc == NDC - 1),
            )
        dst_ap = dst[:, eb, sc * 512:(sc + 1) * 512]
        nc.vector.tensor_scalar_add(dst_ap, ps, bc[:, eb:eb + 1])

    def emit_v_proj(sb):
        ps = pp.tile([P, 512], f32, tag="pp")
        pv = ps[:, :EA]
        for dc in range(NDC):
            nc.tensor.matmul(
                pv,
                lhsT=xT[:, sb, dc, :],
                rhs=wv_sb[:, dc, :],
                start=(dc == 0),
                stop=(dc == NDC - 1),
            )
        # bias (incl. the 1.0 of each head's ones column) fused into the
        # PSUM->SBUF move as a partition-broadcast add
        nc.vector.tensor_add(v_sb[:, sb, :], pv, bv_sb)

    # ---- phase A slice: chunk 0 / pair 0 prerequisites only ----
    def phase_a_slice1():
        for sb in range(4):
            emit_xt(sb, 0)
            emit_xt(sb, 1)
        emit_qk_proj(0, 0, 0)
        emit_qk_proj(1, 0, 0)
        for sb in range(4):
            emit_v_proj(sb)

    # Remaining projection work, drip-fed into the attention phase.
    def fq(which, eb, sc):
        return lambda: emit_qk_proj(which, eb, sc)

    def fxt(sb, dcg):
        return lambda: emit_xt(sb, dcg)

    def fv(sb):
        return lambda: emit_v_proj(sb)

    # Filler placement is as-late-as-possible so the big late attention
    # units aren't starved of PE work: xt(sb) is due just before the
    # qk-projection that reads it; fv(sb) is due just before PV key-block
    # sb of pair 0's chunk sb//4 (v is consumed one whole chunk later
    # than xT, so the v-projections ride one unit later than their xt's).
    prep = {
        (0, 0): [fq(0, 1, 0), fq(1, 1, 0), fxt(4, 0), fxt(4, 1), fxt(5, 0)],
        (1, 0): [fxt(5, 1), fxt(6, 0), fxt(6, 1), fxt(7, 0), fxt(7, 1),
                 fq(0, 0, 1), fq(1, 0, 1)],
        (0, 1): [fq(0, 1, 1), fq(1, 1, 1), fv(4), fv(5), fv(6), fv(7),
                 fxt(8, 0), fxt(8, 1)],
        (1, 1): [fxt(9, 0), fxt(9, 1), fxt(10, 0), fxt(10, 1),
                 fxt(11, 0), fxt(11, 1), fq(0, 0, 2), fq(1, 0, 2)],
        (0, 2): [fq(0, 1, 2), fq(1, 1, 2), fv(8), fv(9), fv(10), fv(11),
                 fxt(12, 0), fxt(12, 1)],
        (1, 2): [fxt(13, 0), fxt(13, 1), fxt(14, 0), fxt(14, 1),
                 fxt(15, 0), fxt(15, 1), fq(0, 0, 3), fq(1, 0, 3)],
        (0, 3): [fq(0, 1, 3), fq(1, 1, 3), fv(12), fv(13), fv(14), fv(15)],
        (1, 3): [],
    }

    # ---- attention: one (head-pair, sq-chunk) unit ----
    def attn_pair_chunk(pair, c):
        eb = pair
        lo, hi = c * CW, (c + 1) * CW
        nj = hi // P
        my_prep = prep[(pair, c)]
        consumed = [0]
        is_last_unit = (pair == 1 and c == NCH - 1)
        nxt = (1, c) if pair == 0 else (0, c + 1)

        def drain_paced(jl):
            tgt = min(-(-len(my_prep) * (jl + 1) // nj), len(my_prep))
            while consumed[0] < tgt:
                my_prep[consumed[0]]()
                consumed[0] += 1

        pacc = paccp.tile([65, 2, CW], f32, tag="pacc")

        def emit_pv(j, at, off, w):
            for h01 in (0, 1):
                h = 2 * pair + h01
                nc.tensor.matmul(
                    pacc[:, h01, off:off + w],
                    lhsT=v_sb[:, j, h * 65:(h + 1) * 65],
                    rhs=at[:, h01, off:off + w],
                    start=(j == 0),
                    stop=(j == nj - 1),
                )

        # tail: one evacuation of the finished accumulator at unit end.
        # (Per-block early tails read pacc banks mid-accumulation, which
        # serializes against every following PV matmul into those banks —
        # the bank-aware overlap tracker turned each one into a ~0.5us PE
        # stall. Reading once after stop=True removes those stalls; the
        # next unit's first PV is deferred anyway, covering the copy.)
        otn = otnp.tile([65, 2, CW], bf16, tag="otn")

        def finish_block(il):
            # transpose + normalize + stage one 128-query block out of the
            # (already evacuated) otn staging tile
            i = c * (CW // P) + il
            for h01 in (0, 1):
                h = 2 * pair + h01
                pot = pp.tile([P, 65], bf16, tag="pp")
                nc.tensor.transpose(
                    pot, otn[:, h01, il * P:(il + 1) * P],
                    ident_sb[0:65, 0:65],
                )
                linv = otnp.tile([P, 1], f32, tag="linv")
                nc.vector.reciprocal(linv, pot[:, DK:DK + 1])
                nc.vector.tensor_scalar_mul(
                    out_sb[:, i, h * DK:(h + 1) * DK], pot[:, 0:DK], linv
                )
            if pair == 1:
                nc.sync.dma_start(
                    out=out_d[i * P:(i + 1) * P, :], in_=out_sb[:, i, :]
                )

        def emit_unit_tail():
            # evacuate the finished accumulator now (frees the pacc banks
            # for the next unit's PV), but DEFER the per-block transpose/
            # normalize/DMA finish-work into the next unit's filler list:
            # 8 back-to-back pot transposes otherwise monopolize the
            # 2-slot pp pool exactly when the next unit's first filler
            # chains want it (~0.7us PE stall per unit boundary).
            nc.vector.tensor_copy(otn, pacc)
            if is_last_unit:
                for il in range(4):
                    finish_block(il)
            else:
                prep[nxt][0:0] = [
                    (lambda il=il: finish_block(il)) for il in range(4)
                ]

        pending = None  # (j, at, off, w): PV deferred one iteration
        for j in range(nj):
            ko = j * P
            sb0 = max(ko, lo)
            off = sb0 - lo
            w = hi - sb0
            # bf16 runs full rate at any width: no >=256 padding needed,
            # diagonal pieces use their exact widths
            ps = pscp.tile([P, 2, CW], f32, tag="sc")
            for h01 in (0, 1):
                po = DK * h01
                # the pair's two matmuls hit disjoint PE row-groups
                # (partitions 0:64 / 64:128) -> they pack concurrently
                nc.tensor.matmul(
                    ps[:, h01, off:off + w],
                    lhsT=kT[po:po + DK, eb, ko:ko + P],
                    rhs=qT[po:po + DK, eb, sb0:sb0 + w],
                    start=True,
                    stop=True,
                )
            at = attnp.tile([P, 2, CW], bf16, tag="at")
            nc.scalar.activation(
                out=at[:, :, off:off + w], in_=ps[:, :, off:off + w],
                func=Exp, scale=0.125,
            )
            if ko >= lo:
                for h01 in (0, 1):
                    nc.gpsimd.tensor_mul(
                        at[:, h01, off:off + P], at[:, h01, off:off + P],
                        mask_sb,
                    )
            drain_paced(j)
            if pending is not None:
                emit_pv(*pending)
            pending = (j, at, off, w)
        emit_pv(*pending)
        emit_unit_tail()

    phase_a_slice1()
    for c in range(NCH):
        for pair in (0, 1):
            attn_pair_chunk(pair, c)


def _prep_core_inputs(inputs, c):
    x = np.asarray(inputs["x"], dtype=np.float32)
    b, hg = c // HPC, c % HPC
    e0 = hg * E

    wq = np.asarray(inputs["Wq"], dtype=np.float32)
    wk = np.asarray(inputs["Wk"], dtype=np.float32)
    wv = np.asarray(inputs["Wv"], dtype=np.float32)
    bq = np.asarray(inputs["bq"], dtype=np.float32)
    bk = np.asarray(inputs["bk"], dtype=np.float32)
    bv = np.asarray(inputs["bv"], dtype=np.float32)

    wq_t = np.ascontiguousarray(wq[e0:e0 + E, :].T)          # [D, E]
    wk_t = np.ascontiguousarray(wk[e0:e0 + E, :].T)
    wv_t = np.zeros((D, EA), dtype=np.float32)
    bv_a = np.zeros((1, EA), dtype=np.float32)
    for lh in range(HPC):
        cols = slice(lh * 65, lh * 65 + DK)
        rows = slice(e0 + lh * DK, e0 + lh * DK + DK)
        wv_t[:, cols] = wv[rows, :].T
        bv_a[0, cols] = bv[rows]
        bv_a[0, lh * 65 + DK] = 1.0                          # ones column
    return {
        "x": np.ascontiguousarray(x[b]).astype(BF),
        "wq_t": wq_t.astype(BF),
        "wk_t": wk_t.astype(BF),
        "wv_t": wv_t.astype(BF),
        "bq": np.ascontiguousarray(bq[e0:e0 + E])[None, :],
        "bk": np.ascontiguousarray(bk[e0:e0 + E])[None, :],
        "bv": np.ascontiguousarray(np.tile(bv_a, (P, 1))),
    }


def kernel(**inputs):
    from concourse.bass_utils import run_bass_kernel_spmd

    if "nc" not in _cache:
        _cache["nc"] = _build_module()
    nc = _cache["nc"]

    in_maps = [_prep_core_inputs(inputs, c) for c in range(NCORES)]
    res = run_bass_kernel_spmd(nc, in_maps, core_ids=list(range(NCORES)))

    out = np.empty((B, S, D), dtype=np.float32)
    for c in range(NCORES):
        b, hg = c // HPC, c % HPC
        out[b, :, hg * E:(hg + 1) * E] = res.results[c]["out"].astype(
            np.float32)
    return out
